# revision 12
# baseline (speedup 1.0000x reference)
"""Trainium2 Bass kernel for ExodusNet: per-timestep 32->1 dense, ExpLeak scan,
LIF (SingleSpike + MembraneSubtract) over T=100.

Contract: kernel(x, w) takes FULL inputs
    x: (32768, 2, 4, 4, 100) f32, w: (1, 32) f32
returns FULL output (32768, 1, 100) f32 (the spike trains).

Sharding: pure data parallel over the batch dim across 8 NeuronCores
(4096 batches per core), w replicated.

Per-core plan (v2 — DMA-roofline oriented):
  - batch decomposition b = h*256 + k2*128 + p (16 half-supertile chunks
    of 2 k-tiles each; p = partition).  One 3.3MB DMA per chunk
    (12.8KB contiguous per (p, k2) pair -> HBM line rate), 4-deep
    buffer pool so the stream never stalls.
  - weighted = sum_f w[f] * x[:,f,:]:
      * features 0..21 on TensorE: stationary diagonals w[c]*I_128,
        fp32 matmuls accumulating in PSUM [128, 200].
      * features 22..31 on VectorE: per-partition-scalar MACs.
      * one tensor_tensor add combines the partials.
  - ExpLeak: tensor_tensor_scan along t per k-tile (DVE);
    (1-alpha) is pre-folded into w on the host, so the scan output u
    is exactly the LIF drive (1-alpha)*syn.
  - LIF runs on the GpSimd/Pool engine in 3 batch groups (k-tiles
    0..15 / 16..27 / 28..31).  Each group's 200-op serial chain starts
    as soon as its last chunk's scan lands, so groups 0 and 1 hide
    entirely under the DMA stream; only group 2 (4 k-tiles) remains as
    a ~8us tail.  v = alpha*v + u_t; s = (v >= 1); v -= s, carried as
    ym = s - v so each step is 2 scalar_tensor_tensor ops.
  - Spikes: pre-reset potentials staged t-major per group, bulk
    (v >= 1) on GpSimd, then per-group contiguous DMA out on the
    scalar-engine queue (doesn't head-of-line-block the x stream).

reps > 1 wraps the whole pipeline in a tc.For_i hardware loop (which
barriers + resets semaphores between iterations), so a single small
NEFF can run hundreds of reps: wall(reps=R) - wall(reps=1) isolates HW
time from host/compile/transfer overhead with high SNR.
"""

import numpy as np
from contextlib import ExitStack

import jax
import concourse.bass as bass
import concourse.bacc as bacc
import concourse.mybir as mybir
from concourse import tile

N_CORES = 8
B_FULL = 32768
BS = B_FULL // N_CORES  # 4096 batches per core
T = 100
F = 32
F_PE = 25          # features done on TensorE (fp32 diag matmuls)
COLS = BS // 128 * T  # 3200 staging/output columns per partition

# x stream chunks (k-tile start, k-tile width): 14 double-k-tile chunks,
# then 4 single k-tiles so the final chunk's compute tail is half-size.
CHUNKS = [(2 * i, 2) for i in range(13)] + [(26 + i, 1) for i in range(6)]

# LIF batch groups: (k0, W, out col base, last chunk index). Each group's
# 200-op serial chain launches right after its last chunk's scan, so all
# but the final (2 k-tile) group hide under the DMA stream.
GROUPS = [
    (0, 16, 0, 7),
    (16, 10, 1600, 12),
    (26, 1, 2600, 13),
    (27, 1, 2700, 14),
    (28, 1, 2800, 15),
    (29, 1, 2900, 16),
    (30, 1, 3000, 17),
    (31, 1, 3100, 18),
]

ALPHA = float(np.exp(-1.0 / 10.0))
ONE_MINUS_ALPHA = float(1.0 - np.exp(-1.0 / 10.0))
THR = 1.0

_DT = mybir.dt.float32
_IN_NAMES = ["x", "wsel", "wb"]


def _build_program(reps: int = 1) -> bass.Bass:
    nc = bacc.Bacc()
    x_in = nc.declare_dram_parameter("x", [BS, 2, 4, 4, T], _DT, isOutput=False)
    # host-precomputed stationary weights: wsel[c] = (1-alpha) * w[0, c] * I_128
    ws_in = nc.declare_dram_parameter("wsel", [F_PE, 128, 128], _DT, isOutput=False)
    # w broadcast across partitions: wb[p, f] = (1-alpha) * w[0, f]
    wb_in = nc.declare_dram_parameter("wb", [128, F], _DT, isOutput=False)
    out = nc.declare_dram_parameter("out", [128, COLS], _DT, isOutput=True)

    # x viewed as [p, k, (f t)] -- per (p, k): 12.8KB contiguous in HBM
    xs = x_in.rearrange("(k p) c2 hh w t -> p k (c2 hh w t)", k=32, p=128)

    mm = mybir.AluOpType.mult
    ad = mybir.AluOpType.add
    ge = mybir.AluOpType.is_ge
    sb = mybir.AluOpType.subtract

    with ExitStack() as ctx:
        tc = ctx.enter_context(tile.TileContext(nc))
        singles = ctx.enter_context(tc.tile_pool(name="singles", bufs=1))
        xpool = ctx.enter_context(tc.tile_pool(name="xpool", bufs=6))
        upool = ctx.enter_context(tc.tile_pool(name="upool", bufs=3))
        psum = ctx.enter_context(tc.tile_pool(name="psum", bufs=4, space="PSUM"))

        # weights/consts load on the scalar-engine DMA ring so the x stream
        # (SP ring) starts immediately
        wsel = singles.tile([128, F_PE * 128], _DT)
        wv = wsel.rearrange("p (c m) -> p c m", c=F_PE)
        nc.scalar.dma_start(out=wv, in_=ws_in.rearrange("c p m -> p c m"))
        wb = singles.tile([128, F], _DT)
        nc.scalar.dma_start(out=wb, in_=wb_in[:, :])

        alphas = singles.tile([128, T], _DT)
        nc.vector.memset(alphas, ALPHA)

        # per-group staging: u (LIF drive, k-major), s (pre-reset v then
        # spikes, t-major), ym (s - v carry)
        ug = [
            singles.tile([128, w * T], _DT, name=f"u{i}")
            for i, (_, w, _, _) in enumerate(GROUPS)
        ]
        sg = [
            singles.tile([128, w * T], _DT, name=f"s{i}")
            for i, (_, w, _, _) in enumerate(GROUPS)
        ]
        ymg = [
            singles.tile([128, w], _DT, name=f"ym{i}")
            for i, (_, w, _, _) in enumerate(GROUPS)
        ]

        def body():
            for g in range(len(GROUPS)):
                nc.vector.memset(ymg[g], 0.0)

            for h, (ks, kw) in enumerate(CHUNKS):
                xh = xpool.tile([128, 2 * F * T], _DT)
                xv = xh.rearrange("p (k ct) -> p k ct", k=2)[:, :kw, :]
                nc.sync.dma_start(out=xv, in_=xs[:, ks : ks + kw, :])

                # TensorE: features 0..F_PE-1 accumulate into PSUM
                pt = psum.tile([128, 2 * T], _DT)
                pts = pt[:, : kw * T]
                for c in range(F_PE):
                    nc.tensor.matmul(
                        pts,
                        wv[:, c, :],
                        xv[:, :, T * c : T * (c + 1)],
                        start=(c == 0),
                        stop=(c == F_PE - 1),
                        tile_position=(0, 0),
                    )

                # VectorE: features F_PE..31 accumulate into upart
                upart = upool.tile([128, 2 * T], _DT)
                ups = upart[:, : kw * T]
                nc.vector.tensor_scalar(
                    ups,
                    xv[:, :, T * F_PE : T * (F_PE + 1)],
                    wb[:, F_PE : F_PE + 1],
                    None,
                    mm,
                )
                for c in range(F_PE + 1, F):
                    nc.vector.scalar_tensor_tensor(
                        out=ups,
                        in0=xv[:, :, T * c : T * (c + 1)],
                        scalar=wb[:, c : c + 1],
                        in1=ups,
                        op0=mm,
                        op1=ad,
                    )
                nc.vector.tensor_tensor(ups, ups, pts, ad)

                # ExpLeak scan per k-tile into this chunk's group staging
                g = next(
                    gi
                    for gi, (k0, w, _cb, _hl) in enumerate(GROUPS)
                    if k0 <= ks < k0 + w
                )
                k0, W, cb, h_last = GROUPS[g]
                for k2 in range(kw):
                    kl = ks + k2 - k0
                    nc.vector.tensor_tensor_scan(
                        out=ug[g][:, T * kl : T * (kl + 1)],
                        data0=alphas,
                        data1=upart[:, T * k2 : T * (k2 + 1)],
                        initial=0.0,
                        op0=mm,
                        op1=ad,
                    )

                # group complete -> LIF chain + spike extract + store
                if h == h_last:
                    uv = ug[g].rearrange("p (k t) -> p k t", t=T)
                    sv = sg[g].rearrange("p (t k) -> p t k", k=W)
                    for t in range(T):
                        nc.vector.scalar_tensor_tensor(
                            out=sv[:, t, :],
                            in0=ymg[g],
                            scalar=-ALPHA,
                            in1=uv[:, :, t],
                            op0=mm,
                            op1=ad,
                        )
                        nc.vector.scalar_tensor_tensor(
                            out=ymg[g],
                            in0=sv[:, t, :],
                            scalar=THR,
                            in1=sv[:, t, :],
                            op0=ge,
                            op1=sb,
                        )
                    for j in range(0, W * T, 400):
                        jj = min(j + 400, W * T)
                        nc.vector.tensor_scalar(
                            sg[g][:, j : jj],
                            sg[g][:, j : jj],
                            THR,
                            None,
                            ge,
                        )
                    nc.scalar.dma_start(
                        out=out[:, cb : cb + W * T], in_=sg[g]
                    )

        if reps == 1:
            body()
        else:
            with tc.For_i(0, reps):
                body()

    nc.finalize()
    return nc


def _build_dma_probe(reps: int = 1, rings: int = 1) -> bass.Bass:
    """x-stream DMA only: measures achievable HBM->SBUF bandwidth."""
    nc = bacc.Bacc()
    x_in = nc.declare_dram_parameter("x", [BS, 2, 4, 4, T], _DT, isOutput=False)
    out = nc.declare_dram_parameter("out", [128, 64], _DT, isOutput=True)
    xs = x_in.rearrange("(k p) c2 hh w t -> p k (c2 hh w t)", k=32, p=128)
    ring = [nc.sync, nc.scalar, nc.vector, nc.gpsimd]

    with ExitStack() as ctx:
        tc = ctx.enter_context(tile.TileContext(nc))
        xpool = ctx.enter_context(tc.tile_pool(name="xpool", bufs=4))

        def body():
            tiles = []
            for h in range(16):
                xh = xpool.tile([128, 2 * F * T], _DT)
                xv = xh.rearrange("p (k2 ct) -> p k2 ct", k2=2)
                ring[h % rings].dma_start(out=xv, in_=xs[:, 2 * h : 2 * h + 2, :])
                tiles.append(xh)
            for j in range(4):
                nc.sync.dma_start(
                    out=out[:, j * 16 : (j + 1) * 16], in_=tiles[-4 + j][:, :16]
                )

        if reps == 1:
            body()
        else:
            with tc.For_i(0, reps):
                body()

    nc.finalize()
    return nc


class _Launcher:
    """Compiled SPMD launcher (mirrors bass2jax.run_bass_via_pjrt but keeps
    the jitted executable so repeat calls don't recompile)."""

    def __init__(self, nc: bass.Bass, donate: bool = True):
        from jax.experimental.shard_map import shard_map
        from jax.sharding import Mesh, PartitionSpec
        from concourse.bass2jax import (
            _bass_exec_p,
            install_neuronx_cc_hook,
            partition_id_tensor,
        )

        install_neuronx_cc_hook()
        self.nc = nc
        partition_name = (
            nc.partition_id_tensor.name if nc.partition_id_tensor else None
        )
        in_names: list[str] = []
        out_names: list[str] = []
        out_avals: list[jax.core.ShapedArray] = []
        zero_shapes: list[tuple] = []
        for alloc in nc.m.functions[0].allocations:
            if not isinstance(alloc, mybir.MemoryLocationSet):
                continue
            name = alloc.memorylocations[0].name
            if alloc.kind == "ExternalInput":
                if name != partition_name:
                    in_names.append(name)
            elif alloc.kind == "ExternalOutput":
                out_names.append(name)
                shape = tuple(alloc.tensor_shape)
                dtype = mybir.dt.np(alloc.dtype)
                out_avals.append(jax.core.ShapedArray(shape, dtype))
                zero_shapes.append((shape, dtype))
        self.in_names = list(in_names)
        self.out_names = out_names
        self.out_avals = out_avals
        self.zero_shapes = zero_shapes
        n_params = len(in_names)
        all_in_names = list(in_names) + list(out_names)
        if partition_name is not None:
            all_in_names.append(partition_name)

        def _body(*args):
            operands = list(args)
            if partition_name is not None:
                operands.append(partition_id_tensor())
            outs = _bass_exec_p.bind(
                *operands,
                out_avals=tuple(out_avals),
                in_names=tuple(all_in_names),
                out_names=tuple(out_names),
                lowering_input_output_aliases=(),
                sim_require_finite=True,
                sim_require_nnan=True,
                nc=nc,
            )
            return tuple(outs)

        devices = jax.devices()[:N_CORES]
        self.mesh = Mesh(np.asarray(devices), ("core",))
        n_outs = len(out_names)
        donate_argnums = (
            tuple(range(n_params, n_params + n_outs)) if donate else ()
        )
        in_specs = (PartitionSpec("core"),) * (n_params + n_outs)
        out_specs = (PartitionSpec("core"),) * n_outs
        self.sharded = jax.jit(
            shard_map(
                _body,
                mesh=self.mesh,
                in_specs=in_specs,
                out_specs=out_specs,
                check_rep=False,
            ),
            donate_argnums=donate_argnums,
            keep_unused=True,
        )

    def zeros(self):
        return [
            np.zeros((N_CORES * s[0], *s[1:]), d) for (s, d) in self.zero_shapes
        ]

    def __call__(self, concat_inputs):
        out_arrs = self.sharded(*concat_inputs, *self.zeros())
        return [np.asarray(o) for o in out_arrs]


_launchers: dict[tuple, _Launcher] = {}


def _get_launcher(reps: int = 1, donate: bool = True, kind: str = "main") -> _Launcher:
    key = (kind, reps, donate)
    if key not in _launchers:
        builder = {"main": _build_program, "dma1": lambda r: _build_dma_probe(r, 1),
                   "dma2": lambda r: _build_dma_probe(r, 2)}[kind]
        _launchers[key] = _Launcher(builder(reps), donate=donate)
    return _launchers[key]


def _make_wsel(w: np.ndarray) -> np.ndarray:
    ws = np.zeros((F_PE, 128, 128), dtype=np.float32)
    idx = np.arange(128)
    for c in range(F_PE):
        ws[c, idx, idx] = w[0, c]
    return ws


def _unscramble(full_out: np.ndarray) -> np.ndarray:
    # full_out: [8*128, 3200]; per core, group block g at cols
    # [cb, cb+100*W): col = cb + t*W + kl -> spike[(k0+kl)*128 + p, t]
    fo = full_out.reshape(N_CORES, 128, COLS)
    res = np.empty((N_CORES, BS, T), dtype=full_out.dtype)
    for (k0, W, cb, _hl) in GROUPS:
        blk = fo[:, :, cb : cb + T * W].reshape(N_CORES, 128, T, W)
        res[:, k0 * 128 : (k0 + W) * 128, :] = blk.transpose(0, 3, 1, 2).reshape(
            N_CORES, W * 128, T
        )
    return res.reshape(B_FULL, 1, T)


def _prep_inputs(x, w):
    x = np.ascontiguousarray(np.asarray(x, dtype=np.float32))
    w = np.ascontiguousarray(np.asarray(w, dtype=np.float32))
    assert x.shape == (B_FULL, 2, 4, 4, T), x.shape
    assert w.shape == (1, F), w.shape
    wsc = (np.float32(ONE_MINUS_ALPHA) * w).astype(np.float32)
    ws = _make_wsel(wsc)
    ws_rep = np.broadcast_to(ws, (N_CORES, *ws.shape)).reshape(
        N_CORES * F_PE, 128, 128
    )
    wb = np.broadcast_to(wsc[0], (128, F))
    wb_rep = np.broadcast_to(wb, (N_CORES, 128, F)).reshape(N_CORES * 128, F)
    return [
        x,
        np.ascontiguousarray(ws_rep),
        np.ascontiguousarray(wb_rep),
    ]


def run(x, w, reps: int = 1):
    launcher = _get_launcher(reps)
    concat_in = _prep_inputs(x, w)
    # input order must match the BIR ExternalInput declaration order
    assert launcher.in_names == _IN_NAMES, launcher.in_names
    outs = launcher(concat_in)
    return _unscramble(outs[0])


def kernel(x, w):
    return run(x, w, reps=1)


# revision 14
# speedup vs baseline: 1.1776x; 1.1776x over previous
"""Trainium2 Bass kernel for ExodusNet: per-timestep 32->1 dense, ExpLeak scan,
LIF (SingleSpike + MembraneSubtract) over T=100.

Contract: kernel(x, w) takes FULL inputs
    x: (32768, 2, 4, 4, 100) f32, w: (1, 32) f32
returns FULL output (32768, 1, 100) f32 (the spike trains).

Sharding: pure data parallel over the batch dim across 8 NeuronCores
(4096 batches per core), w replicated.

Per-core plan (v2 — DMA-roofline oriented):
  - batch decomposition b = h*256 + k2*128 + p (16 half-supertile chunks
    of 2 k-tiles each; p = partition).  One 3.3MB DMA per chunk
    (12.8KB contiguous per (p, k2) pair -> HBM line rate), 4-deep
    buffer pool so the stream never stalls.
  - weighted = sum_f w[f] * x[:,f,:]:
      * features 0..21 on TensorE: stationary diagonals w[c]*I_128,
        fp32 matmuls accumulating in PSUM [128, 200].
      * features 22..31 on VectorE: per-partition-scalar MACs.
      * one tensor_tensor add combines the partials.
  - ExpLeak: tensor_tensor_scan along t per k-tile (DVE);
    (1-alpha) is pre-folded into w on the host, so the scan output u
    is exactly the LIF drive (1-alpha)*syn.
  - LIF runs on the GpSimd/Pool engine in 3 batch groups (k-tiles
    0..15 / 16..27 / 28..31).  Each group's 200-op serial chain starts
    as soon as its last chunk's scan lands, so groups 0 and 1 hide
    entirely under the DMA stream; only group 2 (4 k-tiles) remains as
    a ~8us tail.  v = alpha*v + u_t; s = (v >= 1); v -= s, carried as
    ym = s - v so each step is 2 scalar_tensor_tensor ops.
  - Spikes: pre-reset potentials staged t-major per group, bulk
    (v >= 1) on GpSimd, then per-group contiguous DMA out on the
    scalar-engine queue (doesn't head-of-line-block the x stream).

reps > 1 wraps the whole pipeline in a tc.For_i hardware loop (which
barriers + resets semaphores between iterations), so a single small
NEFF can run hundreds of reps: wall(reps=R) - wall(reps=1) isolates HW
time from host/compile/transfer overhead with high SNR.
"""

import numpy as np
from contextlib import ExitStack

import jax
import concourse.bass as bass
import concourse.bacc as bacc
import concourse.mybir as mybir
from concourse import tile

N_CORES = 8
B_FULL = 32768
BS = B_FULL // N_CORES  # 4096 batches per core
T = 100
F = 32
F_PE = 26          # features done on TensorE (fp32 diag matmuls)
COLS = BS // 128 * T  # 3200 staging/output columns per partition

# x stream chunks (k-tile start, k-tile width): 14 double-k-tile chunks,
# then 4 single k-tiles so the final chunk's compute tail is half-size.
CHUNKS = [(2 * i, 2) for i in range(14)] + [(28, 1), (29, 1), (30, 1), (31, 1)]

# LIF batch groups: (k0, W, out col base, last chunk index). Each group's
# 200-op serial chain launches right after its last chunk's scan, so all
# but the final (2 k-tile) group hide under the DMA stream.
GROUPS = [
    (0, 16, 0, 7),
    (16, 12, 1600, 13),
    (28, 4, 2800, 17),
]

ALPHA = float(np.exp(-1.0 / 10.0))
ONE_MINUS_ALPHA = float(1.0 - np.exp(-1.0 / 10.0))
THR = 1.0

_DT = mybir.dt.float32
_IN_NAMES = ["x", "wsel", "wb"]


def _build_program(reps: int = 1) -> bass.Bass:
    nc = bacc.Bacc()
    x_in = nc.declare_dram_parameter("x", [BS, 2, 4, 4, T], _DT, isOutput=False)
    # host-precomputed stationary weights: wsel[c] = (1-alpha) * w[0, c] * I_128
    ws_in = nc.declare_dram_parameter("wsel", [F_PE, 128, 128], _DT, isOutput=False)
    # w broadcast across partitions: wb[p, f] = (1-alpha) * w[0, f]
    wb_in = nc.declare_dram_parameter("wb", [128, F], _DT, isOutput=False)
    out = nc.declare_dram_parameter("out", [128, COLS], _DT, isOutput=True)

    # x viewed as [p, k, (f t)] -- per (p, k): 12.8KB contiguous in HBM
    xs = x_in.rearrange("(k p) c2 hh w t -> p k (c2 hh w t)", k=32, p=128)

    mm = mybir.AluOpType.mult
    ad = mybir.AluOpType.add
    ge = mybir.AluOpType.is_ge
    sb = mybir.AluOpType.subtract

    with ExitStack() as ctx:
        tc = ctx.enter_context(tile.TileContext(nc))
        singles = ctx.enter_context(tc.tile_pool(name="singles", bufs=1))
        xpool = ctx.enter_context(tc.tile_pool(name="xpool", bufs=6))
        upool = ctx.enter_context(tc.tile_pool(name="upool", bufs=4))
        psum = ctx.enter_context(tc.tile_pool(name="psum", bufs=6, space="PSUM"))

        # weights/consts load on the scalar-engine DMA ring so the x stream
        # (SP ring) starts immediately
        wsel = singles.tile([128, F_PE * 128], _DT)
        wv = wsel.rearrange("p (c m) -> p c m", c=F_PE)
        nc.scalar.dma_start(out=wv, in_=ws_in.rearrange("c p m -> p c m"))
        wb = singles.tile([128, F], _DT)
        nc.scalar.dma_start(out=wb, in_=wb_in[:, :])

        alphas = singles.tile([128, T], _DT)
        nc.vector.memset(alphas, ALPHA)

        # per-group staging: u (LIF drive, k-major), s (pre-reset v then
        # spikes, t-major), ym (s - v carry)
        ug = [
            singles.tile([128, w * T], _DT, name=f"u{i}")
            for i, (_, w, _, _) in enumerate(GROUPS)
        ]
        sg = [
            singles.tile([128, w * T], _DT, name=f"s{i}")
            for i, (_, w, _, _) in enumerate(GROUPS)
        ]
        ymg = [
            singles.tile([128, w], _DT, name=f"ym{i}")
            for i, (_, w, _, _) in enumerate(GROUPS)
        ]

        def body():
            for g in range(len(GROUPS)):
                nc.vector.memset(ymg[g], 0.0)

            for h, (ks, kw) in enumerate(CHUNKS):
                xh = xpool.tile([128, 2 * F * T], _DT)
                xv = xh.rearrange("p (k ct) -> p k ct", k=2)[:, :kw, :]
                nc.sync.dma_start(out=xv, in_=xs[:, ks : ks + kw, :])

                # TensorE: features 0..F_PE-1 accumulate into PSUM
                pt = psum.tile([128, 2 * T], _DT)
                pts = pt[:, : kw * T]
                for c in range(F_PE):
                    nc.tensor.matmul(
                        pts,
                        wv[:, c, :],
                        xv[:, :, T * c : T * (c + 1)],
                        start=(c == 0),
                        stop=(c == F_PE - 1),
                        tile_position=(0, 0),
                    )

                # VectorE: features F_PE..31 accumulate into upart
                upart = upool.tile([128, 2 * T], _DT)
                ups = upart[:, : kw * T]
                nc.vector.tensor_scalar(
                    ups,
                    xv[:, :, T * F_PE : T * (F_PE + 1)],
                    wb[:, F_PE : F_PE + 1],
                    None,
                    mm,
                )
                for c in range(F_PE + 1, F):
                    nc.vector.scalar_tensor_tensor(
                        out=ups,
                        in0=xv[:, :, T * c : T * (c + 1)],
                        scalar=wb[:, c : c + 1],
                        in1=ups,
                        op0=mm,
                        op1=ad,
                    )
                nc.vector.tensor_tensor(ups, ups, pts, ad)

                # ExpLeak scan per k-tile into this chunk's group staging
                g = next(
                    gi
                    for gi, (k0, w, _cb, _hl) in enumerate(GROUPS)
                    if k0 <= ks < k0 + w
                )
                k0, W, cb, h_last = GROUPS[g]
                for k2 in range(kw):
                    kl = ks + k2 - k0
                    nc.vector.tensor_tensor_scan(
                        out=ug[g][:, T * kl : T * (kl + 1)],
                        data0=alphas,
                        data1=upart[:, T * k2 : T * (k2 + 1)],
                        initial=0.0,
                        op0=mm,
                        op1=ad,
                    )

                # group complete -> LIF chain + spike extract + store
                if h == h_last:
                    uv = ug[g].rearrange("p (k t) -> p k t", t=T)
                    sv = sg[g].rearrange("p (t k) -> p t k", k=W)
                    for t in range(T):
                        nc.vector.scalar_tensor_tensor(
                            out=sv[:, t, :],
                            in0=ymg[g],
                            scalar=-ALPHA,
                            in1=uv[:, :, t],
                            op0=mm,
                            op1=ad,
                        )
                        nc.vector.scalar_tensor_tensor(
                            out=ymg[g],
                            in0=sv[:, t, :],
                            scalar=THR,
                            in1=sv[:, t, :],
                            op0=ge,
                            op1=sb,
                        )
                    for j in range(0, W * T, 400):
                        jj = min(j + 400, W * T)
                        nc.vector.tensor_scalar(
                            sg[g][:, j : jj],
                            sg[g][:, j : jj],
                            THR,
                            None,
                            ge,
                        )
                    nc.scalar.dma_start(
                        out=out[:, cb : cb + W * T], in_=sg[g]
                    )

        if reps == 1:
            body()
        else:
            with tc.For_i(0, reps):
                body()

    nc.finalize()
    return nc


def _build_dma_probe(reps: int = 1, rings: int = 1) -> bass.Bass:
    """x-stream DMA only: measures achievable HBM->SBUF bandwidth."""
    nc = bacc.Bacc()
    x_in = nc.declare_dram_parameter("x", [BS, 2, 4, 4, T], _DT, isOutput=False)
    out = nc.declare_dram_parameter("out", [128, 64], _DT, isOutput=True)
    xs = x_in.rearrange("(k p) c2 hh w t -> p k (c2 hh w t)", k=32, p=128)
    ring = [nc.sync, nc.scalar, nc.vector, nc.gpsimd]

    with ExitStack() as ctx:
        tc = ctx.enter_context(tile.TileContext(nc))
        xpool = ctx.enter_context(tc.tile_pool(name="xpool", bufs=4))

        def body():
            tiles = []
            for h in range(16):
                xh = xpool.tile([128, 2 * F * T], _DT)
                xv = xh.rearrange("p (k2 ct) -> p k2 ct", k2=2)
                ring[h % rings].dma_start(out=xv, in_=xs[:, 2 * h : 2 * h + 2, :])
                tiles.append(xh)
            for j in range(4):
                nc.sync.dma_start(
                    out=out[:, j * 16 : (j + 1) * 16], in_=tiles[-4 + j][:, :16]
                )

        if reps == 1:
            body()
        else:
            with tc.For_i(0, reps):
                body()

    nc.finalize()
    return nc


class _Launcher:
    """Compiled SPMD launcher (mirrors bass2jax.run_bass_via_pjrt but keeps
    the jitted executable so repeat calls don't recompile)."""

    def __init__(self, nc: bass.Bass, donate: bool = True):
        from jax.experimental.shard_map import shard_map
        from jax.sharding import Mesh, PartitionSpec
        from concourse.bass2jax import (
            _bass_exec_p,
            install_neuronx_cc_hook,
            partition_id_tensor,
        )

        install_neuronx_cc_hook()
        self.nc = nc
        partition_name = (
            nc.partition_id_tensor.name if nc.partition_id_tensor else None
        )
        in_names: list[str] = []
        out_names: list[str] = []
        out_avals: list[jax.core.ShapedArray] = []
        zero_shapes: list[tuple] = []
        for alloc in nc.m.functions[0].allocations:
            if not isinstance(alloc, mybir.MemoryLocationSet):
                continue
            name = alloc.memorylocations[0].name
            if alloc.kind == "ExternalInput":
                if name != partition_name:
                    in_names.append(name)
            elif alloc.kind == "ExternalOutput":
                out_names.append(name)
                shape = tuple(alloc.tensor_shape)
                dtype = mybir.dt.np(alloc.dtype)
                out_avals.append(jax.core.ShapedArray(shape, dtype))
                zero_shapes.append((shape, dtype))
        self.in_names = list(in_names)
        self.out_names = out_names
        self.out_avals = out_avals
        self.zero_shapes = zero_shapes
        n_params = len(in_names)
        all_in_names = list(in_names) + list(out_names)
        if partition_name is not None:
            all_in_names.append(partition_name)

        def _body(*args):
            operands = list(args)
            if partition_name is not None:
                operands.append(partition_id_tensor())
            outs = _bass_exec_p.bind(
                *operands,
                out_avals=tuple(out_avals),
                in_names=tuple(all_in_names),
                out_names=tuple(out_names),
                lowering_input_output_aliases=(),
                sim_require_finite=True,
                sim_require_nnan=True,
                nc=nc,
            )
            return tuple(outs)

        devices = jax.devices()[:N_CORES]
        self.mesh = Mesh(np.asarray(devices), ("core",))
        n_outs = len(out_names)
        donate_argnums = (
            tuple(range(n_params, n_params + n_outs)) if donate else ()
        )
        in_specs = (PartitionSpec("core"),) * (n_params + n_outs)
        out_specs = (PartitionSpec("core"),) * n_outs
        self.sharded = jax.jit(
            shard_map(
                _body,
                mesh=self.mesh,
                in_specs=in_specs,
                out_specs=out_specs,
                check_rep=False,
            ),
            donate_argnums=donate_argnums,
            keep_unused=True,
        )

    def zeros(self):
        return [
            np.zeros((N_CORES * s[0], *s[1:]), d) for (s, d) in self.zero_shapes
        ]

    def __call__(self, concat_inputs):
        out_arrs = self.sharded(*concat_inputs, *self.zeros())
        return [np.asarray(o) for o in out_arrs]


_launchers: dict[tuple, _Launcher] = {}


def _get_launcher(reps: int = 1, donate: bool = True, kind: str = "main") -> _Launcher:
    key = (kind, reps, donate)
    if key not in _launchers:
        builder = {"main": _build_program, "dma1": lambda r: _build_dma_probe(r, 1),
                   "dma2": lambda r: _build_dma_probe(r, 2)}[kind]
        _launchers[key] = _Launcher(builder(reps), donate=donate)
    return _launchers[key]


def _make_wsel(w: np.ndarray) -> np.ndarray:
    ws = np.zeros((F_PE, 128, 128), dtype=np.float32)
    idx = np.arange(128)
    for c in range(F_PE):
        ws[c, idx, idx] = w[0, c]
    return ws


def _unscramble(full_out: np.ndarray) -> np.ndarray:
    # full_out: [8*128, 3200]; per core, group block g at cols
    # [cb, cb+100*W): col = cb + t*W + kl -> spike[(k0+kl)*128 + p, t]
    fo = full_out.reshape(N_CORES, 128, COLS)
    res = np.empty((N_CORES, BS, T), dtype=full_out.dtype)
    for (k0, W, cb, _hl) in GROUPS:
        blk = fo[:, :, cb : cb + T * W].reshape(N_CORES, 128, T, W)
        res[:, k0 * 128 : (k0 + W) * 128, :] = blk.transpose(0, 3, 1, 2).reshape(
            N_CORES, W * 128, T
        )
    return res.reshape(B_FULL, 1, T)


def _prep_inputs(x, w):
    x = np.ascontiguousarray(np.asarray(x, dtype=np.float32))
    w = np.ascontiguousarray(np.asarray(w, dtype=np.float32))
    assert x.shape == (B_FULL, 2, 4, 4, T), x.shape
    assert w.shape == (1, F), w.shape
    wsc = (np.float32(ONE_MINUS_ALPHA) * w).astype(np.float32)
    ws = _make_wsel(wsc)
    ws_rep = np.broadcast_to(ws, (N_CORES, *ws.shape)).reshape(
        N_CORES * F_PE, 128, 128
    )
    wb = np.broadcast_to(wsc[0], (128, F))
    wb_rep = np.broadcast_to(wb, (N_CORES, 128, F)).reshape(N_CORES * 128, F)
    return [
        x,
        np.ascontiguousarray(ws_rep),
        np.ascontiguousarray(wb_rep),
    ]


def run(x, w, reps: int = 1):
    launcher = _get_launcher(reps)
    concat_in = _prep_inputs(x, w)
    # input order must match the BIR ExternalInput declaration order
    assert launcher.in_names == _IN_NAMES, launcher.in_names
    outs = launcher(concat_in)
    return _unscramble(outs[0])


def kernel(x, w):
    return run(x, w, reps=1)


# revision 15
# speedup vs baseline: 1.2555x; 1.0662x over previous
"""Trainium2 Bass kernel for ExodusNet: per-timestep 32->1 dense, ExpLeak scan,
LIF (SingleSpike + MembraneSubtract) over T=100.

Contract: kernel(x, w) takes FULL inputs
    x: (32768, 2, 4, 4, 100) f32, w: (1, 32) f32
returns FULL output (32768, 1, 100) f32 (the spike trains).

Sharding: pure data parallel over the batch dim across 8 NeuronCores
(4096 batches per core), w replicated.

Per-core plan (DMA-roofline oriented; HW stream rate is 330 GB/s ->
52.4MB/core = 158.6us, the hard floor):
  - batch decomposition b = k*128 + p; x streams as 14 double-k-tile
    chunks (3.3MB) + 4 single k-tiles (taper, so the final chunk's
    compute tail is half-size).  12.8KB contiguous per (p, k) pair ->
    HBM line rate; 6-deep buffer pool so the stream never stalls.
  - weighted = sum_f w[f] * x[:,f,:]:
      * features 0..24 on TensorE: stationary diagonals w[c]*I_128,
        fp32 matmuls accumulating in PSUM.
      * features 25..31 on VectorE: per-partition-scalar MACs.
      * one tensor_tensor add combines the partials.
  - ExpLeak: tensor_tensor_scan along t per k-tile (DVE);
    (1-alpha) is pre-folded into w on the host, so the scan output u
    is exactly the LIF drive (1-alpha)*syn.
  - LIF runs on DVE in 3 batch groups (k-tiles 0..15 / 16..29 /
    30..31); each group's serial chain starts when its last chunk's
    scan lands, so the first two mostly hide under the DMA stream.
    v = alpha*v + u_t; s = (v >= 1); v -= s, carried as ym = s - v so
    each step is 2 scalar_tensor_tensor ops (4 ALUs/step, minimal).
    Dependent chains advance at ~200ns/hop on HW, so the last group's
    200-op chain is a ~40us tail -- the structural cost of the
    sequential recurrence.  (Pool/GpSimd cannot run these ops: the
    ISA rejects them; many narrow chains are HW-slower than few wide
    ones, both verified on HW.)
  - Spikes: pre-reset potentials staged t-major per group, bulk
    (v >= 1), then per-group contiguous DMA out on the scalar-engine
    queue (doesn't head-of-line-block the x stream).

reps > 1 wraps the whole pipeline in a tc.For_i hardware loop (which
barriers + resets semaphores between iterations), so a single small
NEFF can run hundreds of reps: wall(reps=R) - wall(reps=1) isolates HW
time from host/compile/transfer overhead with high SNR.
"""

import numpy as np
from contextlib import ExitStack

import jax
import concourse.bass as bass
import concourse.bacc as bacc
import concourse.mybir as mybir
from concourse import tile

N_CORES = 8
B_FULL = 32768
BS = B_FULL // N_CORES  # 4096 batches per core
T = 100
F = 32
F_PE = 25          # features done on TensorE (fp32 diag matmuls)
COLS = BS // 128 * T  # 3200 staging/output columns per partition

# x stream chunks (k-tile start, k-tile width): 14 double-k-tile chunks,
# then 4 single k-tiles so the final chunk's compute tail is half-size.
CHUNKS = [(2 * i, 2) for i in range(14)] + [(28, 1), (29, 1), (30, 1), (31, 1)]

# LIF batch groups: (k0, W, out col base, last chunk index). Each group's
# 200-op serial chain launches right after its last chunk's scan, so all
# but the final (2 k-tile) group hide under the DMA stream.
GROUPS = [
    (0, 16, 0, 7),
    (16, 14, 1600, 15),
    (30, 2, 3000, 17),
]

ALPHA = float(np.exp(-1.0 / 10.0))
ONE_MINUS_ALPHA = float(1.0 - np.exp(-1.0 / 10.0))
THR = 1.0

_DT = mybir.dt.float32
_IN_NAMES = ["x", "wsel", "wb"]


def _build_program(reps: int = 1) -> bass.Bass:
    nc = bacc.Bacc()
    x_in = nc.declare_dram_parameter("x", [BS, 2, 4, 4, T], _DT, isOutput=False)
    # host-precomputed stationary weights: wsel[c] = (1-alpha) * w[0, c] * I_128
    ws_in = nc.declare_dram_parameter("wsel", [F_PE, 128, 128], _DT, isOutput=False)
    # w broadcast across partitions: wb[p, f] = (1-alpha) * w[0, f]
    wb_in = nc.declare_dram_parameter("wb", [128, F], _DT, isOutput=False)
    out = nc.declare_dram_parameter("out", [128, COLS], _DT, isOutput=True)

    # x viewed as [p, k, (f t)] -- per (p, k): 12.8KB contiguous in HBM
    xs = x_in.rearrange("(k p) c2 hh w t -> p k (c2 hh w t)", k=32, p=128)

    mm = mybir.AluOpType.mult
    ad = mybir.AluOpType.add
    ge = mybir.AluOpType.is_ge
    sb = mybir.AluOpType.subtract

    with ExitStack() as ctx:
        tc = ctx.enter_context(tile.TileContext(nc))
        singles = ctx.enter_context(tc.tile_pool(name="singles", bufs=1))
        xpool = ctx.enter_context(tc.tile_pool(name="xpool", bufs=6))
        upool = ctx.enter_context(tc.tile_pool(name="upool", bufs=3))
        psum = ctx.enter_context(tc.tile_pool(name="psum", bufs=4, space="PSUM"))

        # weights/consts load on the scalar-engine DMA ring so the x stream
        # (SP ring) starts immediately
        wsel = singles.tile([128, F_PE * 128], _DT)
        wv = wsel.rearrange("p (c m) -> p c m", c=F_PE)
        nc.scalar.dma_start(out=wv, in_=ws_in.rearrange("c p m -> p c m"))
        wb = singles.tile([128, F], _DT)
        nc.scalar.dma_start(out=wb, in_=wb_in[:, :])

        alphas = singles.tile([128, T], _DT)
        nc.vector.memset(alphas, ALPHA)

        # per-group staging: u (LIF drive, k-major), s (pre-reset v then
        # spikes, t-major), ym (s - v carry)
        ug = [
            singles.tile([128, w * T], _DT, name=f"u{i}")
            for i, (_, w, _, _) in enumerate(GROUPS)
        ]
        sg = [
            singles.tile([128, w * T], _DT, name=f"s{i}")
            for i, (_, w, _, _) in enumerate(GROUPS)
        ]
        ymg = [
            singles.tile([128, w], _DT, name=f"ym{i}")
            for i, (_, w, _, _) in enumerate(GROUPS)
        ]

        def body():
            for g in range(len(GROUPS)):
                nc.vector.memset(ymg[g], 0.0)

            for h, (ks, kw) in enumerate(CHUNKS):
                xh = xpool.tile([128, 2 * F * T], _DT)
                xv = xh.rearrange("p (k ct) -> p k ct", k=2)[:, :kw, :]
                nc.sync.dma_start(out=xv, in_=xs[:, ks : ks + kw, :])

                # TensorE: features 0..F_PE-1 accumulate into PSUM
                pt = psum.tile([128, 2 * T], _DT)
                pts = pt[:, : kw * T]
                for c in range(F_PE):
                    nc.tensor.matmul(
                        pts,
                        wv[:, c, :],
                        xv[:, :, T * c : T * (c + 1)],
                        start=(c == 0),
                        stop=(c == F_PE - 1),
                        tile_position=(0, 0),
                    )

                # VectorE: features F_PE..31 accumulate into upart
                upart = upool.tile([128, 2 * T], _DT)
                ups = upart[:, : kw * T]
                nc.vector.tensor_scalar(
                    ups,
                    xv[:, :, T * F_PE : T * (F_PE + 1)],
                    wb[:, F_PE : F_PE + 1],
                    None,
                    mm,
                )
                for c in range(F_PE + 1, F):
                    nc.vector.scalar_tensor_tensor(
                        out=ups,
                        in0=xv[:, :, T * c : T * (c + 1)],
                        scalar=wb[:, c : c + 1],
                        in1=ups,
                        op0=mm,
                        op1=ad,
                    )
                nc.vector.tensor_tensor(ups, ups, pts, ad)

                # ExpLeak scan per k-tile into this chunk's group staging
                g = next(
                    gi
                    for gi, (k0, w, _cb, _hl) in enumerate(GROUPS)
                    if k0 <= ks < k0 + w
                )
                k0, W, cb, h_last = GROUPS[g]
                for k2 in range(kw):
                    kl = ks + k2 - k0
                    nc.vector.tensor_tensor_scan(
                        out=ug[g][:, T * kl : T * (kl + 1)],
                        data0=alphas,
                        data1=upart[:, T * k2 : T * (k2 + 1)],
                        initial=0.0,
                        op0=mm,
                        op1=ad,
                    )

                # group complete -> LIF chain + spike extract + store
                if h == h_last:
                    uv = ug[g].rearrange("p (k t) -> p k t", t=T)
                    sv = sg[g].rearrange("p (t k) -> p t k", k=W)
                    for t in range(T):
                        nc.vector.scalar_tensor_tensor(
                            out=sv[:, t, :],
                            in0=ymg[g],
                            scalar=-ALPHA,
                            in1=uv[:, :, t],
                            op0=mm,
                            op1=ad,
                        )
                        nc.vector.scalar_tensor_tensor(
                            out=ymg[g],
                            in0=sv[:, t, :],
                            scalar=THR,
                            in1=sv[:, t, :],
                            op0=ge,
                            op1=sb,
                        )
                    for j in range(0, W * T, 400):
                        jj = min(j + 400, W * T)
                        nc.vector.tensor_scalar(
                            sg[g][:, j : jj],
                            sg[g][:, j : jj],
                            THR,
                            None,
                            ge,
                        )
                    nc.scalar.dma_start(
                        out=out[:, cb : cb + W * T], in_=sg[g]
                    )

        if reps == 1:
            body()
        else:
            with tc.For_i(0, reps):
                body()

    nc.finalize()
    return nc


def _build_dma_probe(reps: int = 1, rings: int = 1) -> bass.Bass:
    """x-stream DMA only: measures achievable HBM->SBUF bandwidth."""
    nc = bacc.Bacc()
    x_in = nc.declare_dram_parameter("x", [BS, 2, 4, 4, T], _DT, isOutput=False)
    out = nc.declare_dram_parameter("out", [128, 64], _DT, isOutput=True)
    xs = x_in.rearrange("(k p) c2 hh w t -> p k (c2 hh w t)", k=32, p=128)
    ring = [nc.sync, nc.scalar, nc.vector, nc.gpsimd]

    with ExitStack() as ctx:
        tc = ctx.enter_context(tile.TileContext(nc))
        xpool = ctx.enter_context(tc.tile_pool(name="xpool", bufs=4))

        def body():
            tiles = []
            for h in range(16):
                xh = xpool.tile([128, 2 * F * T], _DT)
                xv = xh.rearrange("p (k2 ct) -> p k2 ct", k2=2)
                ring[h % rings].dma_start(out=xv, in_=xs[:, 2 * h : 2 * h + 2, :])
                tiles.append(xh)
            for j in range(4):
                nc.sync.dma_start(
                    out=out[:, j * 16 : (j + 1) * 16], in_=tiles[-4 + j][:, :16]
                )

        if reps == 1:
            body()
        else:
            with tc.For_i(0, reps):
                body()

    nc.finalize()
    return nc


class _Launcher:
    """Compiled SPMD launcher (mirrors bass2jax.run_bass_via_pjrt but keeps
    the jitted executable so repeat calls don't recompile)."""

    def __init__(self, nc: bass.Bass, donate: bool = True):
        from jax.experimental.shard_map import shard_map
        from jax.sharding import Mesh, PartitionSpec
        from concourse.bass2jax import (
            _bass_exec_p,
            install_neuronx_cc_hook,
            partition_id_tensor,
        )

        install_neuronx_cc_hook()
        self.nc = nc
        partition_name = (
            nc.partition_id_tensor.name if nc.partition_id_tensor else None
        )
        in_names: list[str] = []
        out_names: list[str] = []
        out_avals: list[jax.core.ShapedArray] = []
        zero_shapes: list[tuple] = []
        for alloc in nc.m.functions[0].allocations:
            if not isinstance(alloc, mybir.MemoryLocationSet):
                continue
            name = alloc.memorylocations[0].name
            if alloc.kind == "ExternalInput":
                if name != partition_name:
                    in_names.append(name)
            elif alloc.kind == "ExternalOutput":
                out_names.append(name)
                shape = tuple(alloc.tensor_shape)
                dtype = mybir.dt.np(alloc.dtype)
                out_avals.append(jax.core.ShapedArray(shape, dtype))
                zero_shapes.append((shape, dtype))
        self.in_names = list(in_names)
        self.out_names = out_names
        self.out_avals = out_avals
        self.zero_shapes = zero_shapes
        n_params = len(in_names)
        all_in_names = list(in_names) + list(out_names)
        if partition_name is not None:
            all_in_names.append(partition_name)

        def _body(*args):
            operands = list(args)
            if partition_name is not None:
                operands.append(partition_id_tensor())
            outs = _bass_exec_p.bind(
                *operands,
                out_avals=tuple(out_avals),
                in_names=tuple(all_in_names),
                out_names=tuple(out_names),
                lowering_input_output_aliases=(),
                sim_require_finite=True,
                sim_require_nnan=True,
                nc=nc,
            )
            return tuple(outs)

        devices = jax.devices()[:N_CORES]
        self.mesh = Mesh(np.asarray(devices), ("core",))
        n_outs = len(out_names)
        donate_argnums = (
            tuple(range(n_params, n_params + n_outs)) if donate else ()
        )
        in_specs = (PartitionSpec("core"),) * (n_params + n_outs)
        out_specs = (PartitionSpec("core"),) * n_outs
        self.sharded = jax.jit(
            shard_map(
                _body,
                mesh=self.mesh,
                in_specs=in_specs,
                out_specs=out_specs,
                check_rep=False,
            ),
            donate_argnums=donate_argnums,
            keep_unused=True,
        )

    def zeros(self):
        return [
            np.zeros((N_CORES * s[0], *s[1:]), d) for (s, d) in self.zero_shapes
        ]

    def __call__(self, concat_inputs):
        out_arrs = self.sharded(*concat_inputs, *self.zeros())
        return [np.asarray(o) for o in out_arrs]


_launchers: dict[tuple, _Launcher] = {}


def _get_launcher(reps: int = 1, donate: bool = True, kind: str = "main") -> _Launcher:
    key = (kind, reps, donate)
    if key not in _launchers:
        builder = {"main": _build_program, "dma1": lambda r: _build_dma_probe(r, 1),
                   "dma2": lambda r: _build_dma_probe(r, 2)}[kind]
        _launchers[key] = _Launcher(builder(reps), donate=donate)
    return _launchers[key]


def _make_wsel(w: np.ndarray) -> np.ndarray:
    ws = np.zeros((F_PE, 128, 128), dtype=np.float32)
    idx = np.arange(128)
    for c in range(F_PE):
        ws[c, idx, idx] = w[0, c]
    return ws


def _unscramble(full_out: np.ndarray) -> np.ndarray:
    # full_out: [8*128, 3200]; per core, group block g at cols
    # [cb, cb+100*W): col = cb + t*W + kl -> spike[(k0+kl)*128 + p, t]
    fo = full_out.reshape(N_CORES, 128, COLS)
    res = np.empty((N_CORES, BS, T), dtype=full_out.dtype)
    for (k0, W, cb, _hl) in GROUPS:
        blk = fo[:, :, cb : cb + T * W].reshape(N_CORES, 128, T, W)
        res[:, k0 * 128 : (k0 + W) * 128, :] = blk.transpose(0, 3, 1, 2).reshape(
            N_CORES, W * 128, T
        )
    return res.reshape(B_FULL, 1, T)


def _prep_inputs(x, w):
    x = np.ascontiguousarray(np.asarray(x, dtype=np.float32))
    w = np.ascontiguousarray(np.asarray(w, dtype=np.float32))
    assert x.shape == (B_FULL, 2, 4, 4, T), x.shape
    assert w.shape == (1, F), w.shape
    wsc = (np.float32(ONE_MINUS_ALPHA) * w).astype(np.float32)
    ws = _make_wsel(wsc)
    ws_rep = np.broadcast_to(ws, (N_CORES, *ws.shape)).reshape(
        N_CORES * F_PE, 128, 128
    )
    wb = np.broadcast_to(wsc[0], (128, F))
    wb_rep = np.broadcast_to(wb, (N_CORES, 128, F)).reshape(N_CORES * 128, F)
    return [
        x,
        np.ascontiguousarray(ws_rep),
        np.ascontiguousarray(wb_rep),
    ]


def run(x, w, reps: int = 1):
    launcher = _get_launcher(reps)
    concat_in = _prep_inputs(x, w)
    # input order must match the BIR ExternalInput declaration order
    assert launcher.in_names == _IN_NAMES, launcher.in_names
    outs = launcher(concat_in)
    return _unscramble(outs[0])


def kernel(x, w):
    return run(x, w, reps=1)


# revision 17
# speedup vs baseline: 1.3523x; 1.0771x over previous
"""Trainium2 Bass kernel for ExodusNet: per-timestep 32->1 dense, ExpLeak scan,
LIF (SingleSpike + MembraneSubtract) over T=100.

Contract: kernel(x, w) takes FULL inputs
    x: (32768, 2, 4, 4, 100) f32, w: (1, 32) f32
returns FULL output (32768, 1, 100) f32 (the spike trains).

Sharding: pure data parallel over the batch dim across 8 NeuronCores
(4096 batches per core), w replicated.

Per-core plan (DMA-roofline oriented; HW stream rate is 330 GB/s ->
52.4MB/core = 158.6us, the hard floor):
  - batch decomposition b = k*128 + p; x streams as 14 double-k-tile
    chunks (3.3MB) + 4 single k-tiles (taper, so the final chunk's
    compute tail is half-size).  12.8KB contiguous per (p, k) pair ->
    HBM line rate; 6-deep buffer pool so the stream never stalls.
  - weighted = sum_f w[f] * x[:,f,:]:
      * features 0..24 on TensorE: stationary diagonals w[c]*I_128,
        fp32 matmuls accumulating in PSUM.
      * features 25..31 on VectorE: per-partition-scalar MACs.
      * one tensor_tensor add combines the partials.
  - ExpLeak: tensor_tensor_scan along t per k-tile (DVE);
    (1-alpha) is pre-folded into w on the host, so the scan output u
    is exactly the LIF drive (1-alpha)*syn.
  - LIF runs on DVE in 3 batch groups (k-tiles 0..15 / 16..29 /
    30..31); each group's serial chain starts when its last chunk's
    scan lands, so the first two mostly hide under the DMA stream.
    v = alpha*v + u_t; s = (v >= 1); v -= s, carried as ym = s - v so
    each step is 2 scalar_tensor_tensor ops (4 ALUs/step, minimal).
    Dependent chains advance at ~200ns/hop on HW, so the last group's
    200-op chain is a ~40us tail -- the structural cost of the
    sequential recurrence.  (Pool/GpSimd cannot run these ops: the
    ISA rejects them; many narrow chains are HW-slower than few wide
    ones, both verified on HW.)
  - Spikes: pre-reset potentials staged t-major per group, bulk
    (v >= 1), then per-group contiguous DMA out on the scalar-engine
    queue (doesn't head-of-line-block the x stream).

reps > 1 wraps the whole pipeline in a tc.For_i hardware loop (which
barriers + resets semaphores between iterations), so a single small
NEFF can run hundreds of reps: wall(reps=R) - wall(reps=1) isolates HW
time from host/compile/transfer overhead with high SNR.
"""

import numpy as np
from contextlib import ExitStack

import jax
import concourse.bass as bass
import concourse.bacc as bacc
import concourse.mybir as mybir
from concourse import tile

N_CORES = 8
B_FULL = 32768
BS = B_FULL // N_CORES  # 4096 batches per core
T = 100
F = 32
F_PE = 25          # features done on TensorE (fp32 diag matmuls)
COLS = BS // 128 * T  # 3200 staging/output columns per partition

# x stream chunks (k-tile start, k-tile width): 14 double-k-tile chunks,
# then 4 single k-tiles so the final chunk's compute tail is half-size.
CHUNKS = [(2 * i, 2) for i in range(14)] + [(28, 1), (29, 1), (30, 1), (31, 1)]

# LIF batch groups: (k0, W, out col base, last chunk index). Each group's
# 200-op serial chain launches right after its last chunk's scan, so all
# but the final (2 k-tile) group hide under the DMA stream.
GROUPS = [
    (0, 16, 0, 7),
    (16, 14, 1600, 15),
    (30, 2, 3000, 17),
]

ALPHA = float(np.exp(-1.0 / 10.0))
ONE_MINUS_ALPHA = float(1.0 - np.exp(-1.0 / 10.0))
THR = 1.0

_DT = mybir.dt.float32
_IN_NAMES = ["x", "wsel", "wb"]


def _build_program(reps: int = 1) -> bass.Bass:
    nc = bacc.Bacc()
    x_in = nc.declare_dram_parameter("x", [BS, 2, 4, 4, T], _DT, isOutput=False)
    # host-precomputed stationary weights: wsel[c] = (1-alpha) * w[0, c] * I_128
    ws_in = nc.declare_dram_parameter("wsel", [F_PE, 128, 128], _DT, isOutput=False)
    # w broadcast across partitions: wb[p, f] = (1-alpha) * w[0, f]
    wb_in = nc.declare_dram_parameter("wb", [128, F], _DT, isOutput=False)
    out = nc.declare_dram_parameter("out", [128, COLS], _DT, isOutput=True)

    # x viewed as [p, k, (f t)] -- per (p, k): 12.8KB contiguous in HBM
    xs = x_in.rearrange("(k p) c2 hh w t -> p k (c2 hh w t)", k=32, p=128)

    mm = mybir.AluOpType.mult
    ad = mybir.AluOpType.add
    ge = mybir.AluOpType.is_ge
    sb = mybir.AluOpType.subtract

    with ExitStack() as ctx:
        tc = ctx.enter_context(tile.TileContext(nc))
        singles = ctx.enter_context(tc.tile_pool(name="singles", bufs=1))
        xpool = ctx.enter_context(tc.tile_pool(name="xpool", bufs=6))
        upool = ctx.enter_context(tc.tile_pool(name="upool", bufs=3))
        psum = ctx.enter_context(tc.tile_pool(name="psum", bufs=4, space="PSUM"))

        # weights/consts load on the scalar-engine DMA ring so the x stream
        # (SP ring) starts immediately
        wsel = singles.tile([128, F_PE * 128], _DT)
        wv = wsel.rearrange("p (c m) -> p c m", c=F_PE)
        nc.scalar.dma_start(out=wv, in_=ws_in.rearrange("c p m -> p c m"))
        wb = singles.tile([128, F], _DT)
        nc.scalar.dma_start(out=wb, in_=wb_in[:, :])

        alphas = singles.tile([128, T], _DT)
        nc.vector.memset(alphas, ALPHA)

        # per-group staging: u (LIF drive, k-major), s (pre-reset v then
        # spikes, t-major), ym (s - v carry)
        ug = [
            singles.tile([128, w * T], _DT, name=f"u{i}")
            for i, (_, w, _, _) in enumerate(GROUPS)
        ]
        sg = [
            singles.tile([128, w * T], _DT, name=f"s{i}")
            for i, (_, w, _, _) in enumerate(GROUPS)
        ]
        ymg = [
            singles.tile([128, w], _DT, name=f"ym{i}")
            for i, (_, w, _, _) in enumerate(GROUPS)
        ]

        def body():
            for g in range(len(GROUPS)):
                nc.vector.memset(ymg[g], 0.0)

            for h, (ks, kw) in enumerate(CHUNKS):
                xh = xpool.tile([128, 2 * F * T], _DT)
                xv = xh.rearrange("p (k ct) -> p k ct", k=2)[:, :kw, :]
                nc.sync.dma_start(out=xv, in_=xs[:, ks : ks + kw, :])

                # TensorE: features 0..F_PE-1 accumulate into PSUM
                pt = psum.tile([128, 2 * T], _DT)
                pts = pt[:, : kw * T]
                for c in range(F_PE):
                    nc.tensor.matmul(
                        pts,
                        wv[:, c, :],
                        xv[:, :, T * c : T * (c + 1)],
                        start=(c == 0),
                        stop=(c == F_PE - 1),
                        tile_position=(0, 0),
                    )

                # VectorE: features F_PE..31 accumulate into upart
                upart = upool.tile([128, 2 * T], _DT)
                ups = upart[:, : kw * T]
                nc.vector.tensor_scalar(
                    ups,
                    xv[:, :, T * F_PE : T * (F_PE + 1)],
                    wb[:, F_PE : F_PE + 1],
                    None,
                    mm,
                )
                for c in range(F_PE + 1, F):
                    nc.vector.scalar_tensor_tensor(
                        out=ups,
                        in0=xv[:, :, T * c : T * (c + 1)],
                        scalar=wb[:, c : c + 1],
                        in1=ups,
                        op0=mm,
                        op1=ad,
                    )
                nc.vector.tensor_tensor(ups, ups, pts, ad)

                # ExpLeak scan per k-tile into this chunk's group staging
                g = next(
                    gi
                    for gi, (k0, w, _cb, _hl) in enumerate(GROUPS)
                    if k0 <= ks < k0 + w
                )
                k0, W, cb, h_last = GROUPS[g]
                for k2 in range(kw):
                    kl = ks + k2 - k0
                    nc.vector.tensor_tensor_scan(
                        out=ug[g][:, T * kl : T * (kl + 1)],
                        data0=alphas,
                        data1=upart[:, T * k2 : T * (k2 + 1)],
                        initial=0.0,
                        op0=mm,
                        op1=ad,
                    )

                # group complete -> LIF chain + spike extract + store
                if h == h_last:
                    uv = ug[g].rearrange("p (k t) -> p k t", t=T)
                    sv = sg[g].rearrange("p (t k) -> p t k", k=W)
                    for t in range(T):
                        nc.vector.scalar_tensor_tensor(
                            out=sv[:, t, :],
                            in0=ymg[g],
                            scalar=-ALPHA,
                            in1=uv[:, :, t],
                            op0=mm,
                            op1=ad,
                        )
                        nc.vector.scalar_tensor_tensor(
                            out=ymg[g],
                            in0=sv[:, t, :],
                            scalar=THR,
                            in1=sv[:, t, :],
                            op0=ge,
                            op1=sb,
                        )
                    for j in range(0, W * T, 400):
                        jj = min(j + 400, W * T)
                        nc.vector.tensor_scalar(
                            sg[g][:, j : jj],
                            sg[g][:, j : jj],
                            THR,
                            None,
                            ge,
                        )
                    nc.scalar.dma_start(
                        out=out[:, cb : cb + W * T], in_=sg[g]
                    )

        if reps == 1:
            body()
        else:
            with tc.For_i(0, reps):
                body()

    nc.finalize()
    return nc


def _build_dma_probe(reps: int = 1, rings: int = 1) -> bass.Bass:
    """x-stream DMA only: measures achievable HBM->SBUF bandwidth."""
    nc = bacc.Bacc()
    x_in = nc.declare_dram_parameter("x", [BS, 2, 4, 4, T], _DT, isOutput=False)
    out = nc.declare_dram_parameter("out", [128, 64], _DT, isOutput=True)
    xs = x_in.rearrange("(k p) c2 hh w t -> p k (c2 hh w t)", k=32, p=128)
    ring = [nc.sync, nc.scalar, nc.vector, nc.gpsimd]

    with ExitStack() as ctx:
        tc = ctx.enter_context(tile.TileContext(nc))
        xpool = ctx.enter_context(tc.tile_pool(name="xpool", bufs=4))

        def body():
            tiles = []
            for h in range(16):
                xh = xpool.tile([128, 2 * F * T], _DT)
                xv = xh.rearrange("p (k2 ct) -> p k2 ct", k2=2)
                ring[h % rings].dma_start(out=xv, in_=xs[:, 2 * h : 2 * h + 2, :])
                tiles.append(xh)
            for j in range(4):
                nc.sync.dma_start(
                    out=out[:, j * 16 : (j + 1) * 16], in_=tiles[-4 + j][:, :16]
                )

        if reps == 1:
            body()
        else:
            with tc.For_i(0, reps):
                body()

    nc.finalize()
    return nc


class _Launcher:
    """Compiled SPMD launcher (mirrors bass2jax.run_bass_via_pjrt but keeps
    the jitted executable so repeat calls don't recompile)."""

    def __init__(self, nc: bass.Bass, donate: bool = True):
        from jax.experimental.shard_map import shard_map
        from jax.sharding import Mesh, PartitionSpec
        from concourse.bass2jax import (
            _bass_exec_p,
            install_neuronx_cc_hook,
            partition_id_tensor,
        )

        install_neuronx_cc_hook()
        self.nc = nc
        partition_name = (
            nc.partition_id_tensor.name if nc.partition_id_tensor else None
        )
        in_names: list[str] = []
        out_names: list[str] = []
        out_avals: list[jax.core.ShapedArray] = []
        zero_shapes: list[tuple] = []
        for alloc in nc.m.functions[0].allocations:
            if not isinstance(alloc, mybir.MemoryLocationSet):
                continue
            name = alloc.memorylocations[0].name
            if alloc.kind == "ExternalInput":
                if name != partition_name:
                    in_names.append(name)
            elif alloc.kind == "ExternalOutput":
                out_names.append(name)
                shape = tuple(alloc.tensor_shape)
                dtype = mybir.dt.np(alloc.dtype)
                out_avals.append(jax.core.ShapedArray(shape, dtype))
                zero_shapes.append((shape, dtype))
        self.in_names = list(in_names)
        self.out_names = out_names
        self.out_avals = out_avals
        self.zero_shapes = zero_shapes
        n_params = len(in_names)
        all_in_names = list(in_names) + list(out_names)
        if partition_name is not None:
            all_in_names.append(partition_name)

        def _body(*args):
            operands = list(args)
            if partition_name is not None:
                operands.append(partition_id_tensor())
            outs = _bass_exec_p.bind(
                *operands,
                out_avals=tuple(out_avals),
                in_names=tuple(all_in_names),
                out_names=tuple(out_names),
                lowering_input_output_aliases=(),
                sim_require_finite=True,
                sim_require_nnan=True,
                nc=nc,
            )
            return tuple(outs)

        devices = jax.devices()[:N_CORES]
        self.mesh = Mesh(np.asarray(devices), ("core",))
        n_outs = len(out_names)
        donate_argnums = (
            tuple(range(n_params, n_params + n_outs)) if donate else ()
        )
        in_specs = (PartitionSpec("core"),) * (n_params + n_outs)
        out_specs = (PartitionSpec("core"),) * n_outs
        self.sharded = jax.jit(
            shard_map(
                _body,
                mesh=self.mesh,
                in_specs=in_specs,
                out_specs=out_specs,
                check_rep=False,
            ),
            donate_argnums=donate_argnums,
            keep_unused=True,
        )

    def zeros(self):
        return [
            np.zeros((N_CORES * s[0], *s[1:]), d) for (s, d) in self.zero_shapes
        ]

    def __call__(self, concat_inputs):
        out_arrs = self.sharded(*concat_inputs, *self.zeros())
        return [np.asarray(o) for o in out_arrs]


_launchers: dict[tuple, _Launcher] = {}


def _get_launcher(reps: int = 1, donate: bool = True, kind: str = "main") -> _Launcher:
    key = (kind, reps, donate)
    if key not in _launchers:
        builder = {"main": _build_program, "dma1": lambda r: _build_dma_probe(r, 1),
                   "dma2": lambda r: _build_dma_probe(r, 2)}[kind]
        _launchers[key] = _Launcher(builder(reps), donate=donate)
    return _launchers[key]


def _make_wsel(w: np.ndarray) -> np.ndarray:
    ws = np.zeros((F_PE, 128, 128), dtype=np.float32)
    idx = np.arange(128)
    for c in range(F_PE):
        ws[c, idx, idx] = w[0, c]
    return ws


def _unscramble(full_out: np.ndarray) -> np.ndarray:
    # full_out: [8*128, 3200]; per core, group block g at cols
    # [cb, cb+100*W): col = cb + t*W + kl -> spike[(k0+kl)*128 + p, t]
    fo = full_out.reshape(N_CORES, 128, COLS)
    res = np.empty((N_CORES, BS, T), dtype=full_out.dtype)
    for (k0, W, cb, _hl) in GROUPS:
        blk = fo[:, :, cb : cb + T * W].reshape(N_CORES, 128, T, W)
        res[:, k0 * 128 : (k0 + W) * 128, :] = blk.transpose(0, 3, 1, 2).reshape(
            N_CORES, W * 128, T
        )
    return res.reshape(B_FULL, 1, T)


def _prep_inputs(x, w):
    x = np.ascontiguousarray(np.asarray(x, dtype=np.float32))
    w = np.ascontiguousarray(np.asarray(w, dtype=np.float32))
    assert x.shape == (B_FULL, 2, 4, 4, T), x.shape
    assert w.shape == (1, F), w.shape
    wsc = (np.float32(ONE_MINUS_ALPHA) * w).astype(np.float32)
    ws = _make_wsel(wsc)
    ws_rep = np.broadcast_to(ws, (N_CORES, *ws.shape)).reshape(
        N_CORES * F_PE, 128, 128
    )
    wb = np.broadcast_to(wsc[0], (128, F))
    wb_rep = np.broadcast_to(wb, (N_CORES, 128, F)).reshape(N_CORES * 128, F)
    return [
        x,
        np.ascontiguousarray(ws_rep),
        np.ascontiguousarray(wb_rep),
    ]


def run(x, w, reps: int = 1):
    launcher = _get_launcher(reps)
    concat_in = _prep_inputs(x, w)
    # input order must match the BIR ExternalInput declaration order
    assert launcher.in_names == _IN_NAMES, launcher.in_names
    outs = launcher(concat_in)
    return _unscramble(outs[0])


def kernel(x, w):
    return run(x, w, reps=1)
